# revision 19
# baseline (speedup 1.0000x reference)
"""AdderNet layer (adder2d conv + residual + power activation) on 8 TRN2
NeuronCores, data-parallel over batch (one image per core).

Math: y = x - sum_{c,kh,kw} |x_pad[b,c,i+kh,j+kw] - W[o,c,kh,kw]|;
out = sign(y)|y|^alpha.

Algorithm: |x - w| is approximated (~1.5e-3 end-to-end rel err) by its
piecewise-linear interpolant on M fixed knots s_k:
    |x - w| ~= a(w) + sum_k c_k(w) * |x - s_k|
which is EXACT for x outside the single knot interval containing w. The
hinge features |x - s_k| depend only on x, so the (c, tap, knot)
reduction becomes TensorEngine matmuls against host-precomputed c_k(w)
coefficient matrices. Zero padding is exact: 0 is a knot, and feature
halos hold |s_k|.

Engine plan per core:
  SP   ring: kb/nbv + x lower half DMA; per-chunk output DMAs
  ACT  ring: x upper half DMA; ACT: table preload, feature pairs 2..,
             per-chunk epilogue (-psum - bias)
  DVE:       halo fills, feature pairs 0..1 (tensor_scalar sub+abs_max),
             per-chunk epilogue (+x)
  GpSimd:    coefficient DMA (SWDGE)
  PE:        288 accumulating matmuls, knot-pair outer, two concurrent
             column-strips (tile_position col 0/64 <-> even/odd chunks)
"""

from contextlib import ExitStack

import numpy as np
import ml_dtypes

import concourse.bass as bass
import concourse.mybir as mybir
from concourse.bass_utils import run_bass_kernel_spmd

B, C, O, H, W = 8, 64, 64, 64, 64
K = 3
NCORES = 8
M_KNOTS = 8             # number of hinge knots (0 is forced in)
NP = M_KNOTS // 2       # feature pairs
NP_DVE = 0              # pairs computed on DVE (rest on ACT)
HP, WP = H + 2, W + 2   # padded feature maps
NCHUNK = 8              # pixel chunks of 8 rows x 64 cols = 512
RC = H // NCHUNK        # rows per chunk

F32 = mybir.dt.float32
BF16 = mybir.dt.bfloat16
AF = mybir.ActivationFunctionType
ALU = mybir.AluOpType


def _make_knots(weight):
    wmin = float(weight.min()) - 1e-4
    wmax = float(weight.max()) + 1e-4
    n_neg = M_KNOTS // 2
    n_pos = M_KNOTS - n_neg - 1
    knots = np.concatenate([
        np.linspace(wmin, 0.0, n_neg + 1)[:-1],
        [0.0],
        np.linspace(0.0, wmax, n_pos + 1)[1:],
    ])
    return knots.astype(np.float64)


def _pl_coeffs(w_flat, knots):
    """Coefficients of the PL interpolant of |x-w| on the knots:
    |x-w| ~= alpha(w) + sum_k C[w,k] |x - s_k|  (end slopes -1/+1)."""
    s = knots
    v = np.abs(s[None, :] - w_flat[:, None])                    # [nw, m]
    interior = (v[:, 1:] - v[:, :-1]) / (s[1:] - s[:-1])[None, :]
    ones = np.ones((len(w_flat), 1))
    slopes = np.concatenate([-ones, interior, ones], axis=1)    # [nw, m+1]
    Cc = (slopes[:, 1:] - slopes[:, :-1]) / 2.0                 # [nw, m]
    al = v[:, 0] - (Cc * np.abs(s[0] - s)[None, :]).sum(1)      # [nw]
    return Cc, al


def _host_prep(weight):
    knots = _make_knots(weight)
    Cc, al = _pl_coeffs(weight.reshape(-1).astype(np.float64), knots)
    Cc = Cc.reshape(O, C, K, K, M_KNOTS)
    al = al.reshape(O, C, K, K)

    # G[p, pair, tap, o] with p = f*64 + c, knot index = 2*pair + f
    G = np.zeros((128, NP, 9, O), dtype=np.float64)
    for f in range(2):
        for pair in range(NP):
            k = 2 * pair + f
            G[f * 64:(f + 1) * 64, pair, :, :] = (
                Cc[:, :, :, :, k].reshape(O, C, 9).transpose(1, 2, 0))
    G = G.astype(ml_dtypes.bfloat16)

    bias_o = al.sum(axis=(1, 2, 3)).astype(np.float32)          # [O]
    nbv = np.tile(-bias_o, 2).reshape(128, 1)

    cfg = np.zeros((128, 2 * NP + 1), dtype=np.float32)
    for f in range(2):
        for pair in range(NP):
            cfg[f * 64:(f + 1) * 64, pair] = -knots[2 * pair + f]
            cfg[f * 64:(f + 1) * 64, NP + pair] = abs(knots[2 * pair + f])
    cfg[:, 2 * NP] = nbv[:, 0]
    return G, cfg


def _build_graph(alpha_is_one, alpha_val=1.0):
    nc = bass.Bass()
    x_im = nc.declare_dram_parameter("x_im", [C, H, W], BF16, isOutput=False)
    g_in = nc.declare_dram_parameter("g_in", [128, NP, 9, O], BF16, isOutput=False)
    cfg_in = nc.declare_dram_parameter("cfg_in", [128, 2 * NP + 1], F32,
                                       isOutput=False)
    out_ext = nc.declare_dram_parameter("out", [O, H, W], F32, isOutput=True)

    ctx = ExitStack()
    with ctx:
        sb = lambda name, shape, dt: ctx.enter_context(
            nc.sbuf_tensor(name, shape, dt))
        xraw = sb("xraw", [128, H, W], BF16)
        feats = sb("feats", [128, NP, HP, WP], BF16)
        g_sb = sb("g_sb", [128, NP, 9, O], BF16)
        cfg_sb = sb("cfg_sb", [128, 2 * NP + 1], F32)
        kb_sb = cfg_sb[:, 0:NP]
        akb_sb = cfg_sb[:, NP:2 * NP]
        nbv_sb = cfg_sb[:, 2 * NP:2 * NP + 1]
        zcol = sb("zcol", [128, 1], F32)
        actwarm = sb("actwarm", [128, 2], F32)
        tmps = [sb(f"tmp{i}", [128, RC, W], F32) for i in range(4)]
        obs = [sb(f"ob{i}", [128, RC, W], F32) for i in range(4)]
        ps = ctx.enter_context(
            nc.psum_tensor("ps", [128, 4, RC, W], F32))

        xa_sem = ctx.enter_context(nc.semaphore("xa_sem"))
        xb_sem = ctx.enter_context(nc.semaphore("xb_sem"))
        cfg_sem = ctx.enter_context(nc.semaphore("cfg_sem"))
        g_sem = ctx.enter_context(nc.semaphore("g_sem"))
        halo_sem = ctx.enter_context(nc.semaphore("halo_sem"))
        featd_sem = ctx.enter_context(nc.semaphore("featd_sem"))
        feata_sem = ctx.enter_context(nc.semaphore("feata_sem"))
        pe_sem = ctx.enter_context(nc.semaphore("pe_sem"))
        epa_sem = ctx.enter_context(nc.semaphore("epa_sem"))
        ep_sem = ctx.enter_context(nc.semaphore("ep_sem"))
        ep2_sem = ctx.enter_context(nc.semaphore("ep2_sem"))
        dout_sem = ctx.enter_context(nc.semaphore("dout_sem"))
        block = ctx.enter_context(nc.Block())

        @block.sync
        def _(sync):
            sync.dma_start(out=cfg_sb[:, :], in_=cfg_in[:, :]).then_inc(cfg_sem, 16)
            sync.dma_start(out=xraw[0:64, :, :],
                           in_=x_im[:, :, :]).then_inc(xa_sem, 16)
            for idx in range(NCHUNK):
                cp, strip = idx // 2, idx % 2
                r0 = (2 * cp + strip) * RC
                pr = slice(strip * 64, strip * 64 + 64)
                sync.wait_ge(ep_sem if alpha_is_one else ep2_sem, idx + 1)
                sync.dma_start(out=out_ext[:, r0:r0 + RC, :],
                               in_=obs[cp][pr, :, :]).then_inc(dout_sem, 16)
            sync.wait_ge(dout_sem, 16 * NCHUNK)

        @block.gpsimd
        def _(gpsimd):
            pass

        @block.vector
        def _(vector):
            # halo fills: feature halo holds |0 - s_k| = |s_k| (akb)
            vector.wait_ge(cfg_sem, 16)
            for p in range(NP):
                vector.tensor_copy(
                    _rows_halo(feats, p),
                    cfg_sb[:, NP + p:NP + p + 1].to_broadcast([128, 2, WP]))
                inst = vector.tensor_copy(
                    _cols_halo(feats, p),
                    cfg_sb[:, NP + p:NP + p + 1].to_broadcast([128, HP, 2]))
                if p == NP - 1:
                    inst.then_inc(halo_sem, 1)
            for idx in range(NCHUNK):
                cp, strip = idx // 2, idx % 2
                r0 = (2 * cp + strip) * RC
                pr = slice(strip * 64, strip * 64 + 64)
                xwin = xraw[pr, r0:r0 + RC, :]
                vector.wait_ge(epa_sem, idx + 1)
                op = ALU.add if alpha_is_one else ALU.subtract
                vector.tensor_tensor(
                    obs[cp][pr, :, :], tmps[cp][pr, :, :], xwin,
                    op).then_inc(ep_sem, 1)

        @block.scalar
        def _(scalar):
            scalar.dma_start(out=xraw[64:128, :, :],
                             in_=x_im[:, :, :]).then_inc(xb_sem, 16)
            scalar.dma_start(out=g_sb[:, :, :, :],
                             in_=g_in[:, :, :, :]).then_inc(g_sem, 16)
            # dummy Abs -> walrus places ACT_TABLE_LOAD here, overlapping DMAs
            scalar.activation(actwarm[0:1, 0:1], actwarm[0:1, 0:1], AF.Abs,
                              bias=actwarm[0:1, 1:2], scale=1.0)
            scalar.wait_ge(cfg_sem, 16)
            scalar.wait_ge(xa_sem, 16)
            scalar.wait_ge(xb_sem, 16)
            for p in range(NP_DVE, NP):
                scalar.activation(
                    feats[:, p, 1:1 + H, 1:1 + W], xraw[:, :, :], AF.Abs,
                    bias=cfg_sb[:, p:p + 1], scale=1.0).then_inc(feata_sem, 1)
            for idx in range(NCHUNK):
                cp, strip = idx // 2, idx % 2
                pr = slice(strip * 64, strip * 64 + 64)
                psd = ps[pr, cp, :, :]
                scalar.wait_ge(pe_sem, idx + 1)
                scalar.activation(
                    tmps[cp][pr, :, :], psd, AF.Identity,
                    bias=cfg_sb[pr, 2 * NP:2 * NP + 1],
                    scale=(-1.0 if alpha_is_one else 1.0)).then_inc(epa_sem, 1)
            if not alpha_is_one:
                for idx in range(NCHUNK):
                    cp, strip = idx // 2, idx % 2
                    pr = slice(strip * 64, strip * 64 + 64)
                    scalar.wait_ge(ep_sem, idx + 1)
                    scalar.activation(obs[cp][pr, :, :], obs[cp][pr, :, :],
                                      AF.Ln)
                    scalar.activation(obs[cp][pr, :, :], obs[cp][pr, :, :],
                                      AF.Exp, scale=float(alpha_val))
                    scalar.mul(obs[cp][pr, :, :], obs[cp][pr, :, :],
                               -1.0).then_inc(ep2_sem, 1)

        @block.tensor
        def _(tensor):
            tensor.wait_ge(g_sem, 16)
            tensor.wait_ge(halo_sem, 1)
            def emit_mm(p, tap, cp, strip):
                kh, kw = divmod(tap, 3)
                first = (p == 0 and tap == 0)
                last = (p == NP - 1 and tap == 8)
                r0 = (2 * cp + strip) * RC
                mov = feats[:, p, r0 + kh:r0 + kh + RC, kw:kw + W]
                st = g_sb[:, p, tap, :]
                psd = ps[strip * 64:strip * 64 + 64, cp, :, :]
                mm = tensor.matmul(psd, st, mov, start=first, stop=last,
                                   tile_position=(0, strip * 64))
                if last:
                    mm.then_inc(pe_sem, 1)

            for p in range(NP):
                tensor.wait_ge(feata_sem, p - NP_DVE + 1)
                if p < NP - 1:
                    for tap in range(9):
                        for cp in range(4):
                            for strip in range(2):
                                emit_mm(p, tap, cp, strip)
                else:
                    # last pair: finish banks one chunk-pair at a time so the
                    # epilogue/output pipeline overlaps the PE tail
                    for cp in range(4):
                        for tap in range(9):
                            for strip in range(2):
                                emit_mm(p, tap, cp, strip)
    return nc


def _rows_halo(feats, p):
    """AP over rows 0 and HP-1 of feature map p: [128, 2, WP]."""
    base = feats[:, p, :, :]
    return bass.AP(tensor=base.tensor, offset=base.offset,
                   ap=[base.ap[0], [(HP - 1) * WP, 2], [1, WP]])


def _cols_halo(feats, p):
    """AP over cols 0 and WP-1 of feature map p: [128, HP, 2]."""
    base = feats[:, p, :, :]
    return bass.AP(tensor=base.tensor, offset=base.offset,
                   ap=[base.ap[0], [WP, HP], [WP - 1, 2]])


def _run(x, weight, alpha, trace=False):
    x = np.ascontiguousarray(np.asarray(x, dtype=np.float32).astype(ml_dtypes.bfloat16))
    weight = np.asarray(weight, dtype=np.float32)
    alpha_val = float(np.asarray(alpha).reshape(-1)[0])
    alpha_is_one = abs(alpha_val - 1.0) < 1e-12

    G, cfg = _host_prep(weight)
    if not alpha_is_one:
        cfg = cfg.copy()
        cfg[:, 2 * NP] = -cfg[:, 2 * NP]  # device path needs +bias
    nc = _build_graph(alpha_is_one, alpha_val)

    in_maps = [{"x_im": x[i], "g_in": G, "cfg_in": cfg}
               for i in range(NCORES)]
    res = run_bass_kernel_spmd(nc, in_maps, list(range(NCORES)), trace=trace)
    out = np.stack([np.asarray(res.results[i]["out"]) for i in range(NCORES)])
    return out.astype(np.float32), res


def kernel(x, weight, alpha):
    out, _ = _run(x, weight, alpha)
    return out


# revision 20
# speedup vs baseline: 1.1923x; 1.1923x over previous
"""AdderNet layer (adder2d conv + residual + power activation) on 8 TRN2
NeuronCores, data-parallel over batch (one image per core).

Math: y = x - sum_{c,kh,kw} |x_pad[b,c,i+kh,j+kw] - W[o,c,kh,kw]|;
out = sign(y)|y|^alpha.

Algorithm: |x - w| is approximated (~1.5e-3 end-to-end rel err) by its
piecewise-linear interpolant on M fixed knots s_k:
    |x - w| ~= a(w) + sum_k c_k(w) * |x - s_k|
which is EXACT for x outside the single knot interval containing w. The
hinge features |x - s_k| depend only on x, so the (c, tap, knot)
reduction becomes TensorEngine matmuls against host-precomputed c_k(w)
coefficient matrices. Zero padding is exact: 0 is a knot, and feature
halos hold |s_k|.

Engine plan per core:
  SP   ring: kb/nbv + x lower half DMA; per-chunk output DMAs
  ACT  ring: x upper half DMA; ACT: table preload, feature pairs 2..,
             per-chunk epilogue (-psum - bias)
  DVE:       halo fills, feature pairs 0..1 (tensor_scalar sub+abs_max),
             per-chunk epilogue (+x)
  GpSimd:    coefficient DMA (SWDGE)
  PE:        288 accumulating matmuls, knot-pair outer, two concurrent
             column-strips (tile_position col 0/64 <-> even/odd chunks)
"""

from contextlib import ExitStack

import numpy as np
import ml_dtypes

import concourse.bass as bass
import concourse.mybir as mybir
from concourse.bass_utils import run_bass_kernel_spmd

B, C, O, H, W = 8, 64, 64, 64, 64
K = 3
NCORES = 8
M_KNOTS = 6             # number of hinge knots (0 is forced in)
NP = M_KNOTS // 2       # feature pairs
NP_DVE = 0              # pairs computed on DVE (rest on ACT)
HP, WP = H + 2, W + 2   # padded feature maps
NCHUNK = 8              # pixel chunks of 8 rows x 64 cols = 512
RC = H // NCHUNK        # rows per chunk

F32 = mybir.dt.float32
BF16 = mybir.dt.bfloat16
AF = mybir.ActivationFunctionType
ALU = mybir.AluOpType


def _make_knots(weight):
    wmin = float(weight.min()) - 1e-4
    wmax = float(weight.max()) + 1e-4
    n_neg = M_KNOTS // 2
    n_pos = M_KNOTS - n_neg - 1
    knots = np.concatenate([
        np.linspace(wmin, 0.0, n_neg + 1)[:-1],
        [0.0],
        np.linspace(0.0, wmax, n_pos + 1)[1:],
    ])
    return knots.astype(np.float64)


def _pl_coeffs(w_flat, knots):
    """Coefficients of the PL interpolant of |x-w| on the knots:
    |x-w| ~= alpha(w) + sum_k C[w,k] |x - s_k|  (end slopes -1/+1)."""
    s = knots
    v = np.abs(s[None, :] - w_flat[:, None])                    # [nw, m]
    interior = (v[:, 1:] - v[:, :-1]) / (s[1:] - s[:-1])[None, :]
    ones = np.ones((len(w_flat), 1))
    slopes = np.concatenate([-ones, interior, ones], axis=1)    # [nw, m+1]
    Cc = (slopes[:, 1:] - slopes[:, :-1]) / 2.0                 # [nw, m]
    al = v[:, 0] - (Cc * np.abs(s[0] - s)[None, :]).sum(1)      # [nw]
    return Cc, al


def _host_prep(weight):
    knots = _make_knots(weight)
    Cc, al = _pl_coeffs(weight.reshape(-1).astype(np.float64), knots)
    Cc = Cc.reshape(O, C, K, K, M_KNOTS)
    al = al.reshape(O, C, K, K)

    # G[p, pair, tap, o] with p = f*64 + c, knot index = 2*pair + f
    G = np.zeros((128, NP, 9, O), dtype=np.float64)
    for f in range(2):
        for pair in range(NP):
            k = 2 * pair + f
            G[f * 64:(f + 1) * 64, pair, :, :] = (
                Cc[:, :, :, :, k].reshape(O, C, 9).transpose(1, 2, 0))
    G = G.astype(ml_dtypes.bfloat16)

    bias_o = al.sum(axis=(1, 2, 3)).astype(np.float32)          # [O]
    nbv = np.tile(-bias_o, 2).reshape(128, 1)

    cfg = np.zeros((128, 2 * NP + 1), dtype=np.float32)
    for f in range(2):
        for pair in range(NP):
            cfg[f * 64:(f + 1) * 64, pair] = -knots[2 * pair + f]
            cfg[f * 64:(f + 1) * 64, NP + pair] = abs(knots[2 * pair + f])
    cfg[:, 2 * NP] = nbv[:, 0]
    return G, cfg


def _build_graph(alpha_is_one, alpha_val=1.0):
    nc = bass.Bass()
    x_im = nc.declare_dram_parameter("x_im", [C, H, W], BF16, isOutput=False)
    g_in = nc.declare_dram_parameter("g_in", [128, NP, 9, O], BF16, isOutput=False)
    cfg_in = nc.declare_dram_parameter("cfg_in", [128, 2 * NP + 1], F32,
                                       isOutput=False)
    out_ext = nc.declare_dram_parameter("out", [O, H, W], F32, isOutput=True)

    ctx = ExitStack()
    with ctx:
        sb = lambda name, shape, dt: ctx.enter_context(
            nc.sbuf_tensor(name, shape, dt))
        xraw = sb("xraw", [128, H, W], BF16)
        feats = sb("feats", [128, NP, HP, WP], BF16)
        g_sb = sb("g_sb", [128, NP, 9, O], BF16)
        cfg_sb = sb("cfg_sb", [128, 2 * NP + 1], F32)
        kb_sb = cfg_sb[:, 0:NP]
        akb_sb = cfg_sb[:, NP:2 * NP]
        nbv_sb = cfg_sb[:, 2 * NP:2 * NP + 1]
        zcol = sb("zcol", [128, 1], F32)
        actwarm = sb("actwarm", [128, 2], F32)
        tmps = [sb(f"tmp{i}", [128, RC, W], F32) for i in range(4)]
        obs = [sb(f"ob{i}", [128, RC, W], F32) for i in range(4)]
        ps = ctx.enter_context(
            nc.psum_tensor("ps", [128, 4, RC, W], F32))

        xa_sem = ctx.enter_context(nc.semaphore("xa_sem"))
        xb_sem = ctx.enter_context(nc.semaphore("xb_sem"))
        cfg_sem = ctx.enter_context(nc.semaphore("cfg_sem"))
        g_sem = ctx.enter_context(nc.semaphore("g_sem"))
        halo_sem = ctx.enter_context(nc.semaphore("halo_sem"))
        featd_sem = ctx.enter_context(nc.semaphore("featd_sem"))
        feata_sem = ctx.enter_context(nc.semaphore("feata_sem"))
        pe_sem = ctx.enter_context(nc.semaphore("pe_sem"))
        epa_sem = ctx.enter_context(nc.semaphore("epa_sem"))
        ep_sem = ctx.enter_context(nc.semaphore("ep_sem"))
        ep2_sem = ctx.enter_context(nc.semaphore("ep2_sem"))
        dout_sem = ctx.enter_context(nc.semaphore("dout_sem"))
        block = ctx.enter_context(nc.Block())

        @block.sync
        def _(sync):
            sync.dma_start(out=cfg_sb[:, :], in_=cfg_in[:, :]).then_inc(cfg_sem, 16)
            sync.dma_start(out=xraw[0:64, :, :],
                           in_=x_im[:, :, :]).then_inc(xa_sem, 16)
            sync.dma_start(out=g_sb[:, 0:1, :, :],
                           in_=g_in[:, 0:1, :, :]).then_inc(g_sem, 16)
            for idx in range(NCHUNK):
                cp, strip = idx // 2, idx % 2
                r0 = (2 * cp + strip) * RC
                pr = slice(strip * 64, strip * 64 + 64)
                sync.wait_ge(ep_sem if alpha_is_one else ep2_sem, idx + 1)
                sync.dma_start(out=out_ext[:, r0:r0 + RC, :],
                               in_=obs[cp][pr, :, :]).then_inc(dout_sem, 16)
            sync.wait_ge(dout_sem, 16 * NCHUNK)

        @block.gpsimd
        def _(gpsimd):
            pass

        @block.vector
        def _(vector):
            # halo fills: feature halo holds |0 - s_k| = |s_k| (akb)
            vector.wait_ge(cfg_sem, 16)
            for p in range(NP):
                vector.tensor_copy(
                    _rows_halo(feats, p),
                    cfg_sb[:, NP + p:NP + p + 1].to_broadcast([128, 2, WP]))
                inst = vector.tensor_copy(
                    _cols_halo(feats, p),
                    cfg_sb[:, NP + p:NP + p + 1].to_broadcast([128, HP, 2]))
                if p == NP - 1:
                    inst.then_inc(halo_sem, 1)
            for idx in range(NCHUNK):
                cp, strip = idx // 2, idx % 2
                r0 = (2 * cp + strip) * RC
                pr = slice(strip * 64, strip * 64 + 64)
                xwin = xraw[pr, r0:r0 + RC, :]
                vector.wait_ge(epa_sem, idx + 1)
                op = ALU.add if alpha_is_one else ALU.subtract
                vector.tensor_tensor(
                    obs[cp][pr, :, :], tmps[cp][pr, :, :], xwin,
                    op).then_inc(ep_sem, 1)

        @block.scalar
        def _(scalar):
            scalar.dma_start(out=xraw[64:128, :, :],
                             in_=x_im[:, :, :]).then_inc(xb_sem, 16)
            scalar.dma_start(out=g_sb[:, 1:NP, :, :],
                             in_=g_in[:, 1:NP, :, :]).then_inc(g_sem, 16)
            # dummy Abs -> walrus places ACT_TABLE_LOAD here, overlapping DMAs
            scalar.activation(actwarm[0:1, 0:1], actwarm[0:1, 0:1], AF.Abs,
                              bias=actwarm[0:1, 1:2], scale=1.0)
            scalar.wait_ge(cfg_sem, 16)
            scalar.wait_ge(xa_sem, 16)
            scalar.wait_ge(xb_sem, 16)
            for p in range(NP_DVE, NP):
                scalar.activation(
                    feats[:, p, 1:1 + H, 1:1 + W], xraw[:, :, :], AF.Abs,
                    bias=cfg_sb[:, p:p + 1], scale=1.0).then_inc(feata_sem, 1)
            for idx in range(NCHUNK):
                cp, strip = idx // 2, idx % 2
                pr = slice(strip * 64, strip * 64 + 64)
                psd = ps[pr, cp, :, :]
                scalar.wait_ge(pe_sem, idx + 1)
                scalar.activation(
                    tmps[cp][pr, :, :], psd, AF.Identity,
                    bias=cfg_sb[pr, 2 * NP:2 * NP + 1],
                    scale=(-1.0 if alpha_is_one else 1.0)).then_inc(epa_sem, 1)
            if not alpha_is_one:
                for idx in range(NCHUNK):
                    cp, strip = idx // 2, idx % 2
                    pr = slice(strip * 64, strip * 64 + 64)
                    scalar.wait_ge(ep_sem, idx + 1)
                    scalar.activation(obs[cp][pr, :, :], obs[cp][pr, :, :],
                                      AF.Ln)
                    scalar.activation(obs[cp][pr, :, :], obs[cp][pr, :, :],
                                      AF.Exp, scale=float(alpha_val))
                    scalar.mul(obs[cp][pr, :, :], obs[cp][pr, :, :],
                               -1.0).then_inc(ep2_sem, 1)

        @block.tensor
        def _(tensor):
            tensor.wait_ge(g_sem, 32)
            tensor.wait_ge(halo_sem, 1)
            def emit_mm(p, tap, cp, strip):
                kh, kw = divmod(tap, 3)
                first = (p == 0 and tap == 0)
                last = (p == NP - 1 and tap == 8)
                r0 = (2 * cp + strip) * RC
                mov = feats[:, p, r0 + kh:r0 + kh + RC, kw:kw + W]
                st = g_sb[:, p, tap, :]
                psd = ps[strip * 64:strip * 64 + 64, cp, :, :]
                mm = tensor.matmul(psd, st, mov, start=first, stop=last,
                                   tile_position=(0, strip * 64))
                if last:
                    mm.then_inc(pe_sem, 1)

            for p in range(NP):
                tensor.wait_ge(feata_sem, p - NP_DVE + 1)
                if p < NP - 1:
                    for tap in range(9):
                        for cp in range(4):
                            for strip in range(2):
                                emit_mm(p, tap, cp, strip)
                else:
                    # last pair: finish banks one chunk-pair at a time so the
                    # epilogue/output pipeline overlaps the PE tail
                    for cp in range(4):
                        for tap in range(9):
                            for strip in range(2):
                                emit_mm(p, tap, cp, strip)
    return nc


def _rows_halo(feats, p):
    """AP over rows 0 and HP-1 of feature map p: [128, 2, WP]."""
    base = feats[:, p, :, :]
    return bass.AP(tensor=base.tensor, offset=base.offset,
                   ap=[base.ap[0], [(HP - 1) * WP, 2], [1, WP]])


def _cols_halo(feats, p):
    """AP over cols 0 and WP-1 of feature map p: [128, HP, 2]."""
    base = feats[:, p, :, :]
    return bass.AP(tensor=base.tensor, offset=base.offset,
                   ap=[base.ap[0], [WP, HP], [WP - 1, 2]])


def _run(x, weight, alpha, trace=False):
    x = np.ascontiguousarray(np.asarray(x, dtype=np.float32).astype(ml_dtypes.bfloat16))
    weight = np.asarray(weight, dtype=np.float32)
    alpha_val = float(np.asarray(alpha).reshape(-1)[0])
    alpha_is_one = abs(alpha_val - 1.0) < 1e-12

    G, cfg = _host_prep(weight)
    if not alpha_is_one:
        cfg = cfg.copy()
        cfg[:, 2 * NP] = -cfg[:, 2 * NP]  # device path needs +bias
    nc = _build_graph(alpha_is_one, alpha_val)

    in_maps = [{"x_im": x[i], "g_in": G, "cfg_in": cfg}
               for i in range(NCORES)]
    res = run_bass_kernel_spmd(nc, in_maps, list(range(NCORES)), trace=trace)
    out = np.stack([np.asarray(res.results[i]["out"]) for i in range(NCORES)])
    return out.astype(np.float32), res


def kernel(x, weight, alpha):
    out, _ = _run(x, weight, alpha)
    return out


# revision 22
# speedup vs baseline: 1.3249x; 1.1112x over previous
"""AdderNet layer (adder2d conv + residual + power activation) on 8 TRN2
NeuronCores, data-parallel over batch (one image per core).

Math: y = x - sum_{c,kh,kw} |x_pad[b,c,i+kh,j+kw] - W[o,c,kh,kw]|;
out = sign(y)|y|^alpha.

Algorithm: |x - w| is approximated (~1.5e-3 end-to-end rel err) by its
piecewise-linear interpolant on M fixed knots s_k:
    |x - w| ~= a(w) + sum_k c_k(w) * |x - s_k|
which is EXACT for x outside the single knot interval containing w. The
hinge features |x - s_k| depend only on x, so the (c, tap, knot)
reduction becomes TensorEngine matmuls against host-precomputed c_k(w)
coefficient matrices. Zero padding is exact: 0 is a knot, and feature
halos hold |s_k|.

Engine plan per core:
  SP   ring: kb/nbv + x lower half DMA; per-chunk output DMAs
  ACT  ring: x upper half DMA; ACT: table preload, feature pairs 2..,
             per-chunk epilogue (-psum - bias)
  DVE:       halo fills, feature pairs 0..1 (tensor_scalar sub+abs_max),
             per-chunk epilogue (+x)
  GpSimd:    coefficient DMA (SWDGE)
  PE:        288 accumulating matmuls, knot-pair outer, two concurrent
             column-strips (tile_position col 0/64 <-> even/odd chunks)
"""

from contextlib import ExitStack

import numpy as np
import ml_dtypes

import concourse.bass as bass
import concourse.mybir as mybir
from concourse.bass_utils import run_bass_kernel_spmd

B, C, O, H, W = 8, 64, 64, 64, 64
K = 3
NCORES = 8
M_KNOTS = 6             # number of hinge knots (0 is forced in)
NP = M_KNOTS // 2       # feature pairs
NP_DVE = 0              # pairs computed on DVE (rest on ACT)
HP, WP = H + 2, W + 2   # padded feature maps
NCHUNK = 8              # pixel chunks of 8 rows x 64 cols = 512
RC = H // NCHUNK        # rows per chunk

XSUBS = [(0, 17), (17, 33), (33, 49), (49, 64)]  # x row sub-DMA blocks

F32 = mybir.dt.float32
BF16 = mybir.dt.bfloat16
AF = mybir.ActivationFunctionType
ALU = mybir.AluOpType


def _make_knots(weight):
    wmin = float(weight.min()) - 1e-4
    wmax = float(weight.max()) + 1e-4
    n_neg = M_KNOTS // 2
    n_pos = M_KNOTS - n_neg - 1
    knots = np.concatenate([
        np.linspace(wmin, 0.0, n_neg + 1)[:-1],
        [0.0],
        np.linspace(0.0, wmax, n_pos + 1)[1:],
    ])
    return knots.astype(np.float64)


def _pl_coeffs(w_flat, knots):
    """Coefficients of the PL interpolant of |x-w| on the knots:
    |x-w| ~= alpha(w) + sum_k C[w,k] |x - s_k|  (end slopes -1/+1)."""
    s = knots
    v = np.abs(s[None, :] - w_flat[:, None])                    # [nw, m]
    interior = (v[:, 1:] - v[:, :-1]) / (s[1:] - s[:-1])[None, :]
    ones = np.ones((len(w_flat), 1))
    slopes = np.concatenate([-ones, interior, ones], axis=1)    # [nw, m+1]
    Cc = (slopes[:, 1:] - slopes[:, :-1]) / 2.0                 # [nw, m]
    al = v[:, 0] - (Cc * np.abs(s[0] - s)[None, :]).sum(1)      # [nw]
    return Cc, al


def _host_prep(weight):
    knots = _make_knots(weight)
    Cc, al = _pl_coeffs(weight.reshape(-1).astype(np.float64), knots)
    Cc = Cc.reshape(O, C, K, K, M_KNOTS)
    al = al.reshape(O, C, K, K)

    # G[p, pair, tap, o] with p = f*64 + c, knot index = 2*pair + f
    G = np.zeros((128, NP, 9, O), dtype=np.float64)
    for f in range(2):
        for pair in range(NP):
            k = 2 * pair + f
            G[f * 64:(f + 1) * 64, pair, :, :] = (
                Cc[:, :, :, :, k].reshape(O, C, 9).transpose(1, 2, 0))
    G = G.astype(ml_dtypes.bfloat16)

    bias_o = al.sum(axis=(1, 2, 3)).astype(np.float32)          # [O]
    nbv = np.tile(-bias_o, 2).reshape(128, 1)

    cfg = np.zeros((128, 2 * NP + 1), dtype=np.float32)
    for f in range(2):
        for pair in range(NP):
            cfg[f * 64:(f + 1) * 64, pair] = -knots[2 * pair + f]
            cfg[f * 64:(f + 1) * 64, NP + pair] = abs(knots[2 * pair + f])
    cfg[:, 2 * NP] = nbv[:, 0]
    return G, cfg


def _build_graph(knots, alpha_is_one, alpha_val=1.0):
    KNOTS = knots
    nc = bass.Bass()
    x_im = nc.declare_dram_parameter("x_im", [C, H, W], BF16, isOutput=False)
    g_in = nc.declare_dram_parameter("g_in", [128, NP, 9, O], BF16, isOutput=False)
    cfg_in = nc.declare_dram_parameter("cfg_in", [128, 2 * NP + 1], F32,
                                       isOutput=False)
    out_ext = nc.declare_dram_parameter("out", [O, H, W], F32, isOutput=True)

    ctx = ExitStack()
    with ctx:
        sb = lambda name, shape, dt: ctx.enter_context(
            nc.sbuf_tensor(name, shape, dt))
        xraw = sb("xraw", [128, H, W], BF16)
        feats = sb("feats", [128, NP, HP, WP], BF16)
        g_sb = sb("g_sb", [128, NP, 9, O], BF16)
        cfg_sb = sb("cfg_sb", [128, 2 * NP + 1], F32)
        kb_sb = cfg_sb[:, 0:NP]
        akb_sb = cfg_sb[:, NP:2 * NP]
        nbv_sb = cfg_sb[:, 2 * NP:2 * NP + 1]
        zcol = sb("zcol", [128, 1], F32)
        actwarm = sb("actwarm", [128, 2], F32)
        tmps = [sb(f"tmp{i}", [128, RC, W], F32) for i in range(4)]
        obs = [sb(f"ob{i}", [128, RC, W], F32) for i in range(4)]
        ps = ctx.enter_context(
            nc.psum_tensor("ps", [128, 4, RC, W], F32))

        xa_sem = ctx.enter_context(nc.semaphore("xa_sem"))
        xb_sem = ctx.enter_context(nc.semaphore("xb_sem"))
        cfg_sem = ctx.enter_context(nc.semaphore("cfg_sem"))
        g_sem = ctx.enter_context(nc.semaphore("g_sem"))
        g2_sem = ctx.enter_context(nc.semaphore("g2_sem"))
        halo_sem = ctx.enter_context(nc.semaphore("halo_sem"))
        featd_sem = ctx.enter_context(nc.semaphore("featd_sem"))
        feata_sem = ctx.enter_context(nc.semaphore("feata_sem"))
        pe_sem = ctx.enter_context(nc.semaphore("pe_sem"))
        epa_sem = ctx.enter_context(nc.semaphore("epa_sem"))
        ep_sem = ctx.enter_context(nc.semaphore("ep_sem"))
        ep2_sem = ctx.enter_context(nc.semaphore("ep2_sem"))
        dout_sem = ctx.enter_context(nc.semaphore("dout_sem"))
        block = ctx.enter_context(nc.Block())

        @block.sync
        def _(sync):
            sync.dma_start(out=cfg_sb[:, :], in_=cfg_in[:, :]).then_inc(cfg_sem, 16)
            sync.dma_start(out=g_sb[:, 0:1, :, :],
                           in_=g_in[:, 0:1, :, :]).then_inc(g_sem, 16)
            for r0, r1 in XSUBS:
                sync.dma_start(out=xraw[0:64, r0:r1, :],
                               in_=x_im[:, r0:r1, :]).then_inc(xa_sem, 16)
            for idx in range(NCHUNK):
                cp, strip = idx // 2, idx % 2
                r0 = (2 * cp + strip) * RC
                pr = slice(strip * 64, strip * 64 + 64)
                sync.wait_ge(ep_sem if alpha_is_one else ep2_sem, idx + 1)
                sync.dma_start(out=out_ext[:, r0:r0 + RC, :],
                               in_=obs[cp][pr, :, :]).then_inc(dout_sem, 16)
            sync.wait_ge(dout_sem, 16 * NCHUNK)

        @block.gpsimd
        def _(gpsimd):
            pass

        @block.vector
        def _(vector):
            # halo fills: feature halo holds |0 - s_k| = |s_k| (baked constants)
            for p in range(NP):
                for half in range(2):
                    hv = float(abs(KNOTS[2 * p + half]))
                    hp = slice(half * 64, half * 64 + 64)
                    vector.memset(feats[hp, p, 0, :], hv)
                    vector.memset(feats[hp, p, HP - 1, :], hv)
                    vector.memset(feats[hp, p, :, 0], hv)
                    inst = vector.memset(feats[hp, p, :, WP - 1], hv)
                    if p == NP - 1 and half == 1:
                        inst.then_inc(halo_sem, 1)
            for idx in range(NCHUNK):
                cp, strip = idx // 2, idx % 2
                r0 = (2 * cp + strip) * RC
                pr = slice(strip * 64, strip * 64 + 64)
                xwin = xraw[pr, r0:r0 + RC, :]
                vector.wait_ge(epa_sem, idx + 1)
                op = ALU.add if alpha_is_one else ALU.subtract
                vector.tensor_tensor(
                    obs[cp][pr, :, :], tmps[cp][pr, :, :], xwin,
                    op).then_inc(ep_sem, 1)

        @block.scalar
        def _(scalar):
            # dummy Abs -> walrus places ACT_TABLE_LOAD here, overlapping DMAs
            scalar.activation(actwarm[0:1, 0:1], actwarm[0:1, 0:1], AF.Abs,
                              bias=actwarm[0:1, 1:2], scale=1.0)
            for r0, r1 in XSUBS:
                scalar.dma_start(out=xraw[64:128, r0:r1, :],
                                 in_=x_im[:, r0:r1, :]).then_inc(xb_sem, 16)
            scalar.dma_start(out=g_sb[:, 1:NP, :, :],
                             in_=g_in[:, 1:NP, :, :]).then_inc(g2_sem, 16)
            scalar.wait_ge(cfg_sem, 16)
            for p in range(NP):
                for k, (r0, r1) in enumerate(XSUBS):
                    if p == 0:
                        scalar.wait_ge(xa_sem, 16 * (k + 1))
                        scalar.wait_ge(xb_sem, 16 * (k + 1))
                    scalar.activation(
                        feats[:, p, 1 + r0:1 + r1, 1:1 + W],
                        xraw[:, r0:r1, :], AF.Abs,
                        bias=cfg_sb[:, p:p + 1],
                        scale=1.0).then_inc(feata_sem, 1)
            for idx in range(NCHUNK):
                cp, strip = idx // 2, idx % 2
                pr = slice(strip * 64, strip * 64 + 64)
                psd = ps[pr, cp, :, :]
                scalar.wait_ge(pe_sem, idx + 1)
                scalar.activation(
                    tmps[cp][pr, :, :], psd, AF.Identity,
                    bias=cfg_sb[pr, 2 * NP:2 * NP + 1],
                    scale=(-1.0 if alpha_is_one else 1.0)).then_inc(epa_sem, 1)
            if not alpha_is_one:
                for idx in range(NCHUNK):
                    cp, strip = idx // 2, idx % 2
                    pr = slice(strip * 64, strip * 64 + 64)
                    scalar.wait_ge(ep_sem, idx + 1)
                    scalar.activation(obs[cp][pr, :, :], obs[cp][pr, :, :],
                                      AF.Ln)
                    scalar.activation(obs[cp][pr, :, :], obs[cp][pr, :, :],
                                      AF.Exp, scale=float(alpha_val))
                    scalar.mul(obs[cp][pr, :, :], obs[cp][pr, :, :],
                               -1.0).then_inc(ep2_sem, 1)

        @block.tensor
        def _(tensor):
            tensor.wait_ge(halo_sem, 1)
            def emit_mm(p, tap, cp, strip):
                kh, kw = divmod(tap, 3)
                first = (p == 0 and tap == 0)
                last = (p == NP - 1 and tap == 8)
                r0 = (2 * cp + strip) * RC
                mov = feats[:, p, r0 + kh:r0 + kh + RC, kw:kw + W]
                st = g_sb[:, p, tap, :]
                psd = ps[strip * 64:strip * 64 + 64, cp, :, :]
                mm = tensor.matmul(psd, st, mov, start=first, stop=last,
                                   tile_position=(0, strip * 64))
                if last:
                    mm.then_inc(pe_sem, 1)

            for p in range(NP):
                tensor.wait_ge(g_sem if p == 0 else g2_sem, 16)
                for cp in range(4):
                    tensor.wait_ge(feata_sem, 4 * p + cp + 1)
                    for tap in range(9):
                        for strip in range(2):
                            emit_mm(p, tap, cp, strip)
    return nc


def _rows_halo(feats, p):
    """AP over rows 0 and HP-1 of feature map p: [128, 2, WP]."""
    base = feats[:, p, :, :]
    return bass.AP(tensor=base.tensor, offset=base.offset,
                   ap=[base.ap[0], [(HP - 1) * WP, 2], [1, WP]])


def _cols_halo(feats, p):
    """AP over cols 0 and WP-1 of feature map p: [128, HP, 2]."""
    base = feats[:, p, :, :]
    return bass.AP(tensor=base.tensor, offset=base.offset,
                   ap=[base.ap[0], [WP, HP], [WP - 1, 2]])


def _run(x, weight, alpha, trace=False):
    x = np.ascontiguousarray(np.asarray(x, dtype=np.float32).astype(ml_dtypes.bfloat16))
    weight = np.asarray(weight, dtype=np.float32)
    alpha_val = float(np.asarray(alpha).reshape(-1)[0])
    alpha_is_one = abs(alpha_val - 1.0) < 1e-12

    G, cfg = _host_prep(weight)
    if not alpha_is_one:
        cfg = cfg.copy()
        cfg[:, 2 * NP] = -cfg[:, 2 * NP]  # device path needs +bias
    nc = _build_graph(_make_knots(weight), alpha_is_one, alpha_val)

    in_maps = [{"x_im": x[i], "g_in": G, "cfg_in": cfg}
               for i in range(NCORES)]
    res = run_bass_kernel_spmd(nc, in_maps, list(range(NCORES)), trace=trace)
    out = np.stack([np.asarray(res.results[i]["out"]) for i in range(NCORES)])
    return out.astype(np.float32), res


def kernel(x, weight, alpha):
    out, _ = _run(x, weight, alpha)
    return out
